# revision 17
# baseline (speedup 1.0000x reference)
"""Weighted-BCE loss kernel for Trainium2 (8 NeuronCores, SPMD data-parallel).

Reference math (torch-style BCELoss with class-balancing weights):
    n   = len(x), s = sum(gt)
    w0  = n / (2*(n-s)),  w1 = n / (2*s)
    L1  = max(log(x),     -100)
    L0  = max(log1p(-x),  -100)
    loss = mean( where(gt==0, w0, w1) * -(gt*L1 + (1-gt)*L0) )

The weights depend only on the GLOBAL positive count s, so the loss
decomposes into 4 global sums computed shard-locally:
    A = sum(gt * L1),  B = sum(gt * L0u),  C = sum(L0u),  s = sum(gt)
    loss = -( A/(2s) + (C-B)/(2(n-s)) )
L0u is UNclamped log(1-x): x is fp32 in [0,1), so 1-x >= 2^-25 and
log(1-x) >= -17.4 — the -100 clamp can never fire on the L0 branch.
The L1 clamp IS needed (x == 0 -> ACT Ln gives -inf) and rides for free
inside the DVE's fused scalar_tensor_tensor op.

Engine split per 1/8 shard (2M elements as [128 partitions, 16384 free]):
  - gt is narrowed to bf16 on the host (0/1 — exact): 2/3 the DMA bytes
    and a PE-compatible dtype.
  - ScalarE (ACT): exactly two Ln passes per tile, writing bf16; no
    accum_out on ACT (accumulator drains were 11us/run on the baseline).
  - VectorE (DVE): one fused op per tile:
      A += reduce( (lnx max -100) * gt )      [scalar_tensor_tensor]
  - TensorE (PE, otherwise idle) does the remaining three sums as
    matmuls accumulating into PSUM across all tiles:
      B: per 128-col chunk, psB[128,128] += gt_chunk.T @ ln1_chunk;
         the useful products live on the diagonal: B = trace(psB).
      C: psC[1,512] += ones[128,1].T @ ln1_chunk512  (column sums)
      S: psS[1,512] += ones[128,1].T @ gt_chunk512
  - All input DMA on the sync HWDGE ring (gt tile first, then x tile);
    compute engines issue no DMAs; deep buffering (bufs=4) keeps DMA
    descriptors always eligible so the ring runs at the HBM roofline.
  - First/last tiles are small to shrink pipeline ramp and drain.
Host gathers accA + psum images from all 8 cores and finishes the (tiny)
all-reduce + trace + final scalar arithmetic in float64.
"""

import numpy as np
import ml_dtypes
from contextlib import ExitStack

import concourse.bass as bass
import concourse.bacc as bacc
import concourse.mybir as mybir
import concourse.tile as tile
from concourse.alu_op_type import AluOpType
from concourse.bass_utils import run_bass_kernel_spmd

N_TOTAL = 16777216
N_CORES = 8
PER_CORE = N_TOTAL // N_CORES   # 2097152
P = 128
FD = PER_CORE // P              # 16384 free elements per partition
TILE_SIZES = [1024, 4096, 4608, 4608, 1536, 512]   # all multiples of 512
assert sum(TILE_SIZES) == FD
NT = len(TILE_SIZES)
LOG_CLAMP = -100.0

# Optional instrumentation knobs for a driver script (harness never sets them).
TRACE = False
LAST_RESULTS = None

_NC_CACHE = None


def _build():
    f32 = mybir.dt.float32
    bf16 = mybir.dt.bfloat16
    Ln = mybir.ActivationFunctionType.Ln

    nc = bacc.Bacc("TRN2")
    x_in = nc.declare_dram_parameter("x", [P, FD], f32, isOutput=False)
    g_in = nc.declare_dram_parameter("gt", [P, FD], bf16, isOutput=False)
    outA = nc.declare_dram_parameter("outA", [P, NT], f32, isOutput=True)
    outB = nc.declare_dram_parameter("outB", [P, 4 * P], f32, isOutput=True)
    outC = nc.declare_dram_parameter("outC", [1, 512], f32, isOutput=True)
    outS = nc.declare_dram_parameter("outS", [1, 512], f32, isOutput=True)

    with tile.TileContext(nc) as tc, ExitStack() as ctx:
        xp = ctx.enter_context(tc.tile_pool(name="xp", bufs=4))
        gp = ctx.enter_context(tc.tile_pool(name="gp", bufs=4))
        lp = ctx.enter_context(tc.tile_pool(name="lp", bufs=2))
        jp = ctx.enter_context(tc.tile_pool(name="jp", bufs=1))
        accp = ctx.enter_context(tc.tile_pool(name="accp", bufs=1))
        pp = ctx.enter_context(tc.psum_pool(name="pp", bufs=1))

        accA = accp.tile([P, NT], f32)
        ones = accp.tile([P, 1], bf16)
        nc.gpsimd.memset(ones[:], 1.0)

        # B accumulates round-robin over 4 PSUM banks: back-to-back short
        # matmuls into one bank serialize on the accumulate RMW; 4-way
        # banking restores pipelining.  A PSUM bank is 2KB/partition
        # (512 f32) and the matmul start-bit resets the WHOLE bank, so
        # each accumulator must own a full bank ([P, 512], first 128
        # columns used).
        psBs = [pp.tile([P, 512], f32, name=f"psB{j}") for j in range(4)]
        psC = pp.tile([1, 512], f32)
        psS = pp.tile([1, 512], f32)

        nb_total = FD // P     # number of 128-col B chunks overall
        ns_total = FD // 512   # number of 512-col C/S chunks overall
        nb_done = 0
        ns_done = 0
        ncc_done = 0

        off = 0
        for i, tfd in enumerate(TILE_SIZES):
            sl = slice(off, off + tfd)
            off += tfd
            xt = xp.tile([P, tfd], f32, tag="xt")
            gt_t = gp.tile([P, tfd], bf16, tag="gt")
            # single HWDGE ring (sync): gt first so PE's S-sums start early
            nc.sync.dma_start(gt_t[:], g_in[:, sl])
            nc.sync.dma_start(xt[:], x_in[:, sl])

            # S += column sums of gt (PE, ones-stationary) — needs only gt
            for c in range(tfd // 512):
                cs = slice(c * 512, (c + 1) * 512)
                nc.tensor.matmul(psS[:], ones[:], gt_t[:, cs],
                                 start=(ns_done == 0), stop=(ns_done == ns_total - 1))
                ns_done += 1

            ln1 = lp.tile([P, tfd], bf16, tag="ln1")
            nc.scalar.activation(ln1[:], xt[:], Ln, bias=1.0, scale=-1.0)

            # B += gt_chunk.T @ ln1_chunk (diag), C += column sums of ln1
            for c in range(tfd // P):
                cs = slice(c * P, (c + 1) * P)
                bank = nb_done % 4
                nc.tensor.matmul(psBs[bank][:, 0:P], gt_t[:, cs], ln1[:, cs],
                                 start=(nb_done < 4), stop=(nb_done >= nb_total - 4))
                nb_done += 1
            for c in range(tfd // 512):
                cs = slice(c * 512, (c + 1) * 512)
                nc.tensor.matmul(psC[:], ones[:], ln1[:, cs],
                                 start=(ncc_done == 0), stop=(ncc_done == ns_total - 1))
                ncc_done += 1

            lnx = lp.tile([P, tfd], bf16, tag="lnx")
            nc.scalar.activation(lnx[:], xt[:], Ln)
            # A += sum(gt * max(ln x, -100))  (DVE fused op)
            junk = jp.tile([P, tfd], bf16, tag="junk")
            nc.vector.scalar_tensor_tensor(
                junk[:], lnx[:], LOG_CLAMP, gt_t[:],
                AluOpType.max, AluOpType.mult,
                accum_out=accA[:, i : i + 1],
            )

        # drain accumulated psums to SBUF, then DRAM
        sbB = accp.tile([P, 4 * P], f32)
        sbC = accp.tile([1, 512], f32)
        sbS = accp.tile([1, 512], f32)
        for j in range(4):
            nc.scalar.copy(sbB[:, j * P : (j + 1) * P], psBs[j][:, 0:P])
        nc.scalar.copy(sbC[:], psC[:])
        nc.scalar.copy(sbS[:], psS[:])
        nc.sync.dma_start(outB[:], sbB[:])
        nc.sync.dma_start(outC[:], sbC[:])
        nc.sync.dma_start(outS[:], sbS[:])
        nc.sync.dma_start(outA[:], accA[:])

    nc.compile()
    return nc


def get_nc():
    global _NC_CACHE
    if _NC_CACHE is None:
        _NC_CACHE = _build()
    return _NC_CACHE


def make_in_maps(x, gt):
    x = np.ascontiguousarray(np.asarray(x, dtype=np.float32).reshape(-1))
    gt = np.asarray(gt).reshape(-1)
    assert x.shape == (N_TOTAL,) and gt.shape == (N_TOTAL,)
    # narrow the 0/1 labels to bf16 (exact): 2/3 the DMA bytes, PE-compatible
    gtb = np.ascontiguousarray(gt.astype(ml_dtypes.bfloat16))
    in_maps = []
    for c in range(N_CORES):
        sl = slice(c * PER_CORE, (c + 1) * PER_CORE)
        in_maps.append({
            "x": x[sl].reshape(P, FD),
            "gt": gtb[sl].reshape(P, FD),
        })
    return in_maps


def combine(results):
    """All-reduce the per-core partial sums and finish the loss formula."""
    A = B = C = S = 0.0
    for r in results:
        A += r["outA"].astype(np.float64).sum()
        oB = r["outB"].astype(np.float64)
        for j in range(4):
            B += np.trace(oB[:, j * P : (j + 1) * P])
        C += r["outC"].astype(np.float64).sum()
        S += r["outS"].astype(np.float64).sum()
    n = float(N_TOTAL)
    result = -(A / (2.0 * S) + (C - B) / (2.0 * (n - S)))
    return np.array(result, dtype=np.float32)


def kernel(x, gt):
    global LAST_RESULTS
    nc = get_nc()
    in_maps = make_in_maps(x, gt)
    br = run_bass_kernel_spmd(nc, in_maps, list(range(N_CORES)))
    LAST_RESULTS = br
    return combine(br.results)


# revision 18
# speedup vs baseline: 1.3221x; 1.3221x over previous
"""Weighted-BCE loss kernel for Trainium2 (8 NeuronCores, SPMD data-parallel).

Reference math (torch-style BCELoss with class-balancing weights):
    n   = len(x), s = sum(gt)
    w0  = n / (2*(n-s)),  w1 = n / (2*s)
    L1  = max(log(x),     -100)
    L0  = max(log1p(-x),  -100)
    loss = mean( where(gt==0, w0, w1) * -(gt*L1 + (1-gt)*L0) )

The weights depend only on the GLOBAL positive count s, so the loss
decomposes into 4 global sums computed shard-locally:
    A = sum(gt * L1),  B = sum(gt * L0u),  C = sum(L0u),  s = sum(gt)
    loss = -( A/(2s) + (C-B)/(2(n-s)) )
L0u is UNclamped log(1-x): x is fp32 in [0,1), so 1-x >= 2^-25 and
log(1-x) >= -17.4 — the -100 clamp can never fire on the L0 branch.
The L1 clamp IS needed (x == 0 -> ACT Ln gives -inf, measured) and rides
for free inside the DVE's fused scalar_tensor_tensor op.

Engine split per 1/8 shard (2M elements as [128 partitions, 16384 free]):
  - gt is narrowed to bf16 on the host (0/1 — exact): 2/3 the DMA bytes,
    PE-compatible dtype, and 2x-mode DVE operand.
  - ScalarE (ACT): exactly two Ln passes per tile, writing bf16; no
    accum_out on ACT (accumulator drains cost ~11us/run on the baseline).
  - VectorE (DVE), two ops per tile:
      A += reduce( (lnx max -100) * gt )   scalar_tensor_tensor, 1x rate
      prod = gt * ln1                      tensor_tensor, 2x bf16 rate
    (prod is exact: gt is 0/1 so the bf16 product is just a select)
  - TensorE (PE, otherwise idle) turns the remaining sums into matmuls
    against a never-changing ones[128,1] stationary, all at full rate,
    each accumulating into its own PSUM bank across all tiles:
      S[1,512] += ones.T @ gt_chunk512     (column sums of gt)
      C[1,512] += ones.T @ ln1_chunk512
      B[1,512] += ones.T @ prod_chunk512
  - All input DMA on the sync HWDGE ring (x tile first, then gt tile);
    compute engines issue no DMAs; deep buffering (bufs=4) keeps DMA
    descriptors always eligible so the ring runs at the HBM roofline.
  - First/last tiles are small to shrink pipeline ramp and drain.
Host gathers accA + the three [1,512] partial rows from all 8 cores and
finishes the (tiny) all-reduce + final scalar arithmetic in float64.
"""

import numpy as np
import ml_dtypes
from contextlib import ExitStack

import concourse.bass as bass
import concourse.bacc as bacc
import concourse.mybir as mybir
import concourse.tile as tile
from concourse.alu_op_type import AluOpType
from concourse.bass_utils import run_bass_kernel_spmd

N_TOTAL = 16777216
N_CORES = 8
PER_CORE = N_TOTAL // N_CORES   # 2097152
P = 128
FD = PER_CORE // P              # 16384 free elements per partition
TILE_SIZES = [1024, 4096, 4608, 4608, 1536, 512]   # all multiples of 512
assert sum(TILE_SIZES) == FD
NT = len(TILE_SIZES)
LOG_CLAMP = -100.0

# Optional instrumentation knobs for a driver script (harness never sets them).
TRACE = False
LAST_RESULTS = None

_NC_CACHE = None


def _build():
    f32 = mybir.dt.float32
    bf16 = mybir.dt.bfloat16
    Ln = mybir.ActivationFunctionType.Ln

    nc = bacc.Bacc("TRN2")
    x_in = nc.declare_dram_parameter("x", [P, FD], f32, isOutput=False)
    g_in = nc.declare_dram_parameter("gt", [P, FD], bf16, isOutput=False)
    outA = nc.declare_dram_parameter("outA", [P, NT], f32, isOutput=True)
    outB = nc.declare_dram_parameter("outB", [1, 512], f32, isOutput=True)
    outC = nc.declare_dram_parameter("outC", [1, 512], f32, isOutput=True)
    outS = nc.declare_dram_parameter("outS", [1, 512], f32, isOutput=True)

    with tile.TileContext(nc) as tc, ExitStack() as ctx:
        xp = ctx.enter_context(tc.tile_pool(name="xp", bufs=4))
        gp = ctx.enter_context(tc.tile_pool(name="gp", bufs=4))
        lp = ctx.enter_context(tc.tile_pool(name="lp", bufs=2))
        prp = ctx.enter_context(tc.tile_pool(name="prp", bufs=2))
        jp = ctx.enter_context(tc.tile_pool(name="jp", bufs=1))
        accp = ctx.enter_context(tc.tile_pool(name="accp", bufs=1))
        pp = ctx.enter_context(tc.psum_pool(name="pp", bufs=1))

        accA = accp.tile([P, NT], f32)
        ones = accp.tile([P, 1], bf16)
        nc.gpsimd.memset(ones[:], 1.0)

        psB = pp.tile([1, 512], f32)
        psC = pp.tile([1, 512], f32)
        psS = pp.tile([1, 512], f32)

        ns_total = FD // 512   # number of 512-col chunks overall
        done = {"B": 0, "C": 0, "S": 0}

        def reduce_chunks(ps, key, src, tfd):
            for c in range(tfd // 512):
                cs = slice(c * 512, (c + 1) * 512)
                nc.tensor.matmul(ps[:], ones[:], src[:, cs],
                                 start=(done[key] == 0),
                                 stop=(done[key] == ns_total - 1))
                done[key] += 1

        off = 0
        for i, tfd in enumerate(TILE_SIZES):
            sl = slice(off, off + tfd)
            off += tfd
            xt = xp.tile([P, tfd], f32, tag="xt")
            gt_t = gp.tile([P, tfd], bf16, tag="gt")
            # single HWDGE ring (sync): x first — ACT is the longest chain
            nc.sync.dma_start(xt[:], x_in[:, sl])
            nc.sync.dma_start(gt_t[:], g_in[:, sl])

            # S += column sums of gt (PE)
            reduce_chunks(psS, "S", gt_t, tfd)

            ln1 = lp.tile([P, tfd], bf16, tag="ln1")
            nc.scalar.activation(ln1[:], xt[:], Ln, bias=1.0, scale=-1.0)

            # prod = gt * ln1 (DVE tensor_tensor, 2x bf16), then
            # B += column sums of prod (PE);  C += column sums of ln1 (PE)
            prod = prp.tile([P, tfd], bf16, tag="prod")
            nc.vector.tensor_tensor(prod[:], gt_t[:], ln1[:], AluOpType.mult)
            reduce_chunks(psB, "B", prod, tfd)
            reduce_chunks(psC, "C", ln1, tfd)

            lnx = lp.tile([P, tfd], bf16, tag="lnx")
            nc.scalar.activation(lnx[:], xt[:], Ln)
            # A += sum(gt * max(ln x, -100))  (DVE fused op)
            junk = jp.tile([P, tfd], bf16, tag="junk")
            nc.vector.scalar_tensor_tensor(
                junk[:], lnx[:], LOG_CLAMP, gt_t[:],
                AluOpType.max, AluOpType.mult,
                accum_out=accA[:, i : i + 1],
            )

        # drain accumulated psums to SBUF, then DRAM
        sbB = accp.tile([1, 512], f32)
        sbC = accp.tile([1, 512], f32)
        sbS = accp.tile([1, 512], f32)
        nc.scalar.copy(sbB[:], psB[:])
        nc.scalar.copy(sbC[:], psC[:])
        nc.scalar.copy(sbS[:], psS[:])
        nc.sync.dma_start(outB[:], sbB[:])
        nc.sync.dma_start(outC[:], sbC[:])
        nc.sync.dma_start(outS[:], sbS[:])
        nc.sync.dma_start(outA[:], accA[:])

    nc.compile()
    return nc


def get_nc():
    global _NC_CACHE
    if _NC_CACHE is None:
        _NC_CACHE = _build()
    return _NC_CACHE


def make_in_maps(x, gt):
    x = np.ascontiguousarray(np.asarray(x, dtype=np.float32).reshape(-1))
    gt = np.asarray(gt).reshape(-1)
    assert x.shape == (N_TOTAL,) and gt.shape == (N_TOTAL,)
    # narrow the 0/1 labels to bf16 (exact): 2/3 the DMA bytes, PE-compatible
    gtb = np.ascontiguousarray(gt.astype(ml_dtypes.bfloat16))
    in_maps = []
    for c in range(N_CORES):
        sl = slice(c * PER_CORE, (c + 1) * PER_CORE)
        in_maps.append({
            "x": x[sl].reshape(P, FD),
            "gt": gtb[sl].reshape(P, FD),
        })
    return in_maps


def combine(results):
    """All-reduce the per-core partial sums and finish the loss formula."""
    A = B = C = S = 0.0
    for r in results:
        A += r["outA"].astype(np.float64).sum()
        B += r["outB"].astype(np.float64).sum()
        C += r["outC"].astype(np.float64).sum()
        S += r["outS"].astype(np.float64).sum()
    n = float(N_TOTAL)
    result = -(A / (2.0 * S) + (C - B) / (2.0 * (n - S)))
    return np.array(result, dtype=np.float32)


def kernel(x, gt):
    global LAST_RESULTS
    nc = get_nc()
    in_maps = make_in_maps(x, gt)
    br = run_bass_kernel_spmd(nc, in_maps, list(range(N_CORES)))
    LAST_RESULTS = br
    return combine(br.results)
